# revision 34
# baseline (speedup 1.0000x reference)
"""Bass/Tile kernel for nn_MultiHeadAttention (B=2, S=2048, D=1024, H=16).

Sharding: 8 cores = 2 (batch) x 4 (head-chunks of 4 heads).
Each core computes, for its batch b and its 4 heads (2 pairs of 2 heads):
  qpT/kpT = (x @ W{q,k} + b)^T   in [dout, token] bf16 layout
  vp      = v @ Wv + bv          in [token, dout] bf16 layout
  scoresT = kp @ qp^T            per head, [k, q] f32 PSUM
  at      = exp(scoresT) bf16, Z[k] via ACT accum_out
  pv[qh]  = sum_kb (vp/Z)^T @ at  accumulated IN PSUM across all kb
  out[p]  = hcT_p^T @ Wo_p        per-pair bf16 partial (host sums 8 partials)

All matmuls bf16 (rel err ~1.25e-2 vs 2e-2 gate, matches numpy sim exactly).
v3: resident raw q/k/v tiles loaded with 2KB-per-partition DMA lines (2x DMA
efficiency vs 512-token chunks), packed wqkv weight tensor, early-exp pass
(scores for kb 0-3 on the first q-half start right after q-proj chunks 0/1 +
k-proj chunk 0), PV(kb-1) emitted between score groups to keep the PE stream
dense (DVFS ramp needs continuous tensor work), per-pair O-proj overlapped
with the other pair's attention.
"""

import sys

sys.path.insert(0, "/opt/trn_rl_repo")

from contextlib import ExitStack

import numpy as np
import ml_dtypes

import concourse.bass as bass
import concourse.mybir as mybir
import concourse.tile as tile
from concourse import bacc
from concourse.bass_utils import run_bass_kernel_spmd

BF16 = mybir.dt.bfloat16
F32 = mybir.dt.float32
AF = mybir.ActivationFunctionType
ALU = mybir.AluOpType

D = 1024
NK = 8  # k-tiles over D
DOUT = 256  # per-core head dims (4 heads)
NPAIR = 2  # pairs of heads (128 dout each)
HD = 64
S = 2048
B = 2
NKB = S // 128  # k-token blocks
NQH = S // 1024  # 1024-wide q halves
NTC = S // 512  # proj token chunks
NTT = S // 128  # token tiles
EARLY = 4  # kb blocks whose qh=0 scores/exp run during the projection head


def build_kernel():
    nc = bacc.Bacc("TRN2", target_bir_lowering=False, debug=False)

    qT = nc.dram_tensor("qT", [D, S], BF16, kind="ExternalInput")
    kT = nc.dram_tensor("kT", [D, S], BF16, kind="ExternalInput")
    vT = nc.dram_tensor("vT", [D, S], BF16, kind="ExternalInput")
    wqkv = nc.dram_tensor("wqkv", [D, 3 * DOUT], BF16, kind="ExternalInput")
    wo = nc.dram_tensor("wo", [DOUT, D], BF16, kind="ExternalInput")
    bq = nc.dram_tensor("bq", [NPAIR, 128, 1], F32, kind="ExternalInput")
    bk = nc.dram_tensor("bk", [NPAIR, 128, 1], F32, kind="ExternalInput")
    bv = nc.dram_tensor("bv", [DOUT], F32, kind="ExternalInput")
    out = nc.dram_tensor("out", [NPAIR, S, D], BF16, kind="ExternalOutput")

    qTv = qT.ap().rearrange("(t p) s -> t p s", p=128)  # [8, 128, S]
    kTv = kT.ap().rearrange("(t p) s -> t p s", p=128)
    vTv = vT.ap().rearrange("(t p) s -> t p s", p=128)
    wqkvv = wqkv.ap().rearrange("(t p) m -> t p m", p=128)  # [8, 128, 768]
    wov = wo.ap().rearrange("(t p) m -> t p m", p=128)  # [2, 128, 1024]
    bqv = bq.ap().rearrange("a p o -> p a o")  # [128, 2, 1]
    bkv = bk.ap().rearrange("a p o -> p a o")
    outv = out.ap().rearrange("a (t p) m -> a t p m", p=128)  # [2,16,128,1024]

    bv_bcast_ap = bass.AP(tensor=bv.ap().tensor, offset=0, ap=[[0, 128], [1, DOUT]])

    with tile.TileContext(nc) as tc, ExitStack() as ctx:
        sb = ctx.enter_context(tc.tile_pool(name="sb", bufs=1))

        # resident raw inputs + weights
        qT_sb = sb.tile([128, NK, S], BF16, tag="qT")
        kT_sb = sb.tile([128, NK, S], BF16, tag="kT")
        vT_sb = sb.tile([128, NK, S], BF16, tag="vT")
        wqkv_sb = sb.tile([128, NK, 3 * DOUT], BF16, tag="wqkv")
        wo_sb = sb.tile([128, NPAIR, D], BF16, tag="wo")
        bq_sb = sb.tile([128, NPAIR, 1], F32, tag="bq")
        bk_sb = sb.tile([128, NPAIR, 1], F32, tag="bk")
        bv_sb = sb.tile([128, DOUT], F32, tag="bv")

        # projection outputs (resident)
        qpT_sb = sb.tile([128, NPAIR, S], BF16, tag="qpT")
        kpT_sb = sb.tile([128, NPAIR, S], BF16, tag="kpT")
        vp_sb = sb.tile([128, NTT, DOUT], BF16, tag="vp")
        hcT_sb = sb.tile([128, NPAIR, S], BF16, tag="hcT")
        hc_acc = sb.tile([128, S], F32, tag="hc_acc")  # per-pair, reused

        psa = ctx.enter_context(tc.tile_pool(name="ps_all", bufs=1, space="PSUM"))
        asb = ctx.enter_context(tc.tile_pool(name="att_sb", bufs=1))
        osb = ctx.enter_context(tc.tile_pool(name="o_sb", bufs=1))

        # ---- DMA schedule: ordered by first consumption ----
        # wqkv[0] + qA first so the very first matmul can start early.
        for kk in range(NK):  # all weights first: chunk kk-loops never stall
            nc.sync.dma_start(out=wqkv_sb[:, kk, :], in_=wqkvv[kk])
        nc.sync.dma_start(out=bq_sb[:], in_=bqv)
        nc.sync.dma_start(out=bk_sb[:], in_=bkv)
        nc.sync.dma_start(out=bv_sb[:], in_=bv_bcast_ap)
        for kk in range(NK):  # q tokens 0..1023
            nc.sync.dma_start(out=qT_sb[:, kk, 0:1024], in_=qTv[kk][:, 0:1024])
        for kk in range(NK):  # k tokens 0..511 (quarter: first exp path)
            nc.sync.dma_start(out=kT_sb[:, kk, 0:512], in_=kTv[kk][:, 0:512])
        for kk in range(NK):  # k tokens 512..1023
            nc.sync.dma_start(out=kT_sb[:, kk, 512:1024], in_=kTv[kk][:, 512:1024])
        for kk in range(NK):  # q tokens 1024..2047
            nc.sync.dma_start(out=qT_sb[:, kk, 1024:2048], in_=qTv[kk][:, 1024:2048])
        for kk in range(NK):  # v tokens 0..1023
            nc.sync.dma_start(out=vT_sb[:, kk, 0:1024], in_=vTv[kk][:, 0:1024])
        for kk in range(NK):  # k tokens 1024..2047
            nc.sync.dma_start(out=kT_sb[:, kk, 1024:2048], in_=kTv[kk][:, 1024:2048])
        for kk in range(NK):  # v tokens 1024..2047
            nc.sync.dma_start(out=vT_sb[:, kk, 1024:2048], in_=vTv[kk][:, 1024:2048])
        for t in range(NPAIR):
            nc.sync.dma_start(out=wo_sb[:, t, :], in_=wov[t])

        # ---- emission helpers ----
        BIGB = 4  # 'big' PSUM slots: 4 x [128,1024] f32 = all 8 banks

        def emit_qkproj_pair(XT_sb, woff, b_sb, XPT, tci, p):
            tsl = slice(tci * 512, tci * 512 + 512)
            ps_t = psa.tile([128, 512], F32, tag="big", bufs=BIGB, name=f"pj{p}")
            for kk in range(NK):
                nc.tensor.matmul(
                    ps_t[:],
                    lhsT=wqkv_sb[:, kk, woff + p * 128 : woff + p * 128 + 128],
                    rhs=XT_sb[:, kk, tsl],
                    start=(kk == 0),
                    stop=(kk == NK - 1),
                )
            nc.vector.tensor_scalar_add(XPT[:, p, tsl], ps_t[:], b_sb[:, p, :])

        def emit_vproj(tt):
            psv = psa.tile([128, DOUT], F32, tag="big", bufs=BIGB, name="projv")
            for kk in range(NK):
                nc.tensor.matmul(
                    psv[:],
                    lhsT=vT_sb[:, kk, tt * 128 : tt * 128 + 128],
                    rhs=wqkv_sb[:, kk, 2 * DOUT : 3 * DOUT],
                    start=(kk == 0),
                    stop=(kk == NK - 1),
                )
            nc.vector.scalar_tensor_tensor(
                out=vp_sb[:, tt, :],
                in0=psv[:],
                scalar=1.0,
                in1=bv_sb[:],
                op0=ALU.mult,
                op1=ALU.add,
            )

        def emit_oproj(p, tt, tail=False):
            ost = osb.tile([128, D], BF16, tag="ost", bufs=6, name="ost")
            pso = psa.tile([128, D], F32, tag="big", bufs=BIGB, name="o")
            for dc in range(2):
                nc.tensor.matmul(
                    pso[:, dc * 512 : dc * 512 + 512],
                    lhsT=hcT_sb[:, p, tt * 128 : tt * 128 + 128],
                    rhs=wo_sb[:, p, dc * 512 : dc * 512 + 512],
                    start=True,
                    stop=True,
                )
            if tail and tt % 2 == 0:
                nc.scalar.copy(ost[:], pso[:])
            else:
                nc.vector.tensor_copy(ost[:], pso[:])
            nc.sync.dma_start(out=outv[p][tt], in_=ost[:])

        def emit_sc_exp(p, kb, h, qh, z4):
            ksl = slice(kb * 128, kb * 128 + 128)
            hsl = slice(h * 64, h * 64 + 64)
            sc = psa.tile([128, 1024], F32, tag="big", bufs=BIGB, name=f"sc{h}{qh}")
            for qq in range(2):
                qsl = slice(qh * 1024 + qq * 512, qh * 1024 + qq * 512 + 512)
                nc.tensor.matmul(
                    sc[:, qq * 512 : qq * 512 + 512],
                    lhsT=kpT_sb[hsl, p, ksl],
                    rhs=qpT_sb[hsl, p, qsl],
                    start=True,
                    stop=True,
                )
            at = asb.tile([128, 1024], BF16, tag="at", bufs=14, name=f"at{h}{qh}")
            nc.scalar.activation(
                out=at[:], in_=sc[:], func=AF.Exp,
                accum_out=z4[:, h, qh : qh + 1],
            )
            return at

        def emit_zchain_h(p, kb, h, z4, vhs):
            # per-head z so the DVE work for head h issues as soon as that
            # head's two exps land (shortens the exp->vhs->PV ring)
            z1 = asb.tile([128, 1], F32, tag="z2", bufs=8, name="z1")
            nc.vector.tensor_add(z1[:], z4[:, h, 0:1], z4[:, h, 1:2])
            rz = asb.tile([128, 1], F32, tag="rz", bufs=8, name="rz")
            nc.vector.reciprocal(rz[:], z1[:])
            nc.vector.tensor_scalar_mul(
                vhs[:, h, :],
                vp_sb[:, kb, p * 128 + h * 64 : p * 128 + h * 64 + 64],
                rz[:],
            )

        def emit_pv_mm(ats, vhs):
            pvts = []
            for qh in range(NQH):
                pvt = psa.tile([128, 1024], F32, tag="big", bufs=BIGB, name="pvt")
                for h in range(2):
                    for qq in range(2):
                        nc.tensor.matmul(
                            pvt[h * 64 : h * 64 + 64, qq * 512 : qq * 512 + 512],
                            lhsT=vhs[:, h, :],
                            rhs=ats[(h, qh)][:, qq * 512 : qq * 512 + 512],
                            start=True,
                            stop=True,
                            tile_position=(0, h * 64),
                            skip_group_check=True,
                        )
                pvts.append(pvt)
            return pvts

        def emit_pv_acc(kb, pvts):
            # hc accumulation: emitted AFTER the z-chain so DVE's in-order
            # queue never blocks the exp-facing z work behind PE-dependent adds
            for qh in range(NQH):
                qsl = slice(qh * 1024, qh * 1024 + 1024)
                if kb == 0:
                    nc.vector.tensor_copy(hc_acc[:, qsl], pvts[qh][:])
                else:
                    nc.vector.tensor_add(hc_acc[:, qsl], hc_acc[:, qsl], pvts[qh][:])

        # ---- projection head + early exp pass (pair 0, kb<EARLY, qh=0) ----
        # only the chunks the early pass needs come first (pair-0 q chunks
        # 0/1 + pair-0 k chunk 0); every other chunk trickles later.
        emit_qkproj_pair(qT_sb, 0, bq_sb, qpT_sb, 0, 0)
        emit_qkproj_pair(qT_sb, 0, bq_sb, qpT_sb, 1, 0)
        emit_qkproj_pair(kT_sb, DOUT, bk_sb, kpT_sb, 0, 0)

        EARLY_TRICKLE = {
            0: (kT_sb, DOUT, bk_sb, kpT_sb, 1, 0),
            1: (qT_sb, 0, bq_sb, qpT_sb, 2, 0),
            2: (qT_sb, 0, bq_sb, qpT_sb, 3, 0),
            3: (qT_sb, 0, bq_sb, qpT_sb, 0, 1),
        }
        at_early = {}
        z4_early = {}
        for kb in range(EARLY):
            z4 = asb.tile([128, 2, NQH], F32, tag="z4e", bufs=EARLY + 1, name="z4e")
            z4_early[kb] = z4
            for h in range(2):
                at_early[(kb, h)] = emit_sc_exp(0, kb, h, 0, z4)
            emit_qkproj_pair(*EARLY_TRICKLE[kb])
        emit_vproj(0)
        emit_vproj(1)

        # ---- attention main loops ----
        # remaining projection chunks, scheduled just-in-time:
        # pair-0 k chunks c before kb=4c; all pair-1 chunks before iter 16
        MAIN_TRICKLE = {
            0: (qT_sb, 0, bq_sb, qpT_sb, 1, 1),
            1: (kT_sb, DOUT, bk_sb, kpT_sb, 0, 1),
            2: (qT_sb, 0, bq_sb, qpT_sb, 2, 1),
            3: (qT_sb, 0, bq_sb, qpT_sb, 3, 1),
            4: (kT_sb, DOUT, bk_sb, kpT_sb, 2, 0),
            5: (kT_sb, DOUT, bk_sb, kpT_sb, 1, 1),
            6: (kT_sb, DOUT, bk_sb, kpT_sb, 2, 1),
            8: (kT_sb, DOUT, bk_sb, kpT_sb, 3, 0),
            9: (kT_sb, DOUT, bk_sb, kpT_sb, 3, 1),
        }
        # fused loop over (pair, kb): the pair transition pipelines like any
        # other iteration (prev iteration's PV rides through the next one)
        prev = None  # (kb, ats, vhs)
        for it in range(NPAIR * NKB):
            p, kb = it // NKB, it % NKB
            vhs = asb.tile([128, 2, HD], BF16, tag="vhs", bufs=8, name="vhs")
            if p == 0 and kb < EARLY:
                z4 = z4_early[kb]
                ats = {
                    (0, 0): at_early[(kb, 0)],
                    (1, 0): at_early[(kb, 1)],
                }
                ats[(0, 1)] = emit_sc_exp(p, kb, 0, 1, z4)
                emit_zchain_h(p, kb, 0, z4, vhs)
                ats[(1, 1)] = emit_sc_exp(p, kb, 1, 1, z4)
                emit_zchain_h(p, kb, 1, z4, vhs)
            else:
                z4 = asb.tile([128, 2, NQH], F32, tag="z4", bufs=8, name="z4")
                ats = {}
                ats[(0, 0)] = emit_sc_exp(p, kb, 0, 0, z4)
                ats[(0, 1)] = emit_sc_exp(p, kb, 0, 1, z4)
                emit_zchain_h(p, kb, 0, z4, vhs)
                ats[(1, 0)] = emit_sc_exp(p, kb, 1, 0, z4)
                ats[(1, 1)] = emit_sc_exp(p, kb, 1, 1, z4)
                emit_zchain_h(p, kb, 1, z4, vhs)
            if prev is not None:
                pvts = emit_pv_mm(prev[1], prev[2])
                emit_pv_acc(prev[0], pvts)
                if prev[0] == NKB - 1:  # pair-0 finished: stage its hcT
                    for qh in range(NQH):
                        qsl = slice(qh * 1024, qh * 1024 + 1024)
                        nc.vector.tensor_copy(hcT_sb[:, 0, qsl], hc_acc[:, qsl])
            prev = (kb, ats, vhs)
            # trickled, dependency-free PE filler
            if p == 0:
                if kb < 14:
                    emit_vproj(kb + 2)
                tr = MAIN_TRICKLE.get(kb)
                if tr is not None:
                    emit_qkproj_pair(*tr)
            elif kb >= 8:
                # pair-0 O-proj packed into the LAST iterations so the PE
                # enters the tail dense (keeps the DVFS clock up)
                emit_oproj(0, 2 * (kb - 8))
                emit_oproj(0, 2 * (kb - 8) + 1)

        # ---- drain: last PV, pair-1 hcT, remaining O-projection ----
        pvts = emit_pv_mm(prev[1], prev[2])
        emit_pv_acc(NKB - 1, pvts)
        for qh in range(NQH):
            qsl = slice(qh * 1024, qh * 1024 + 1024)
            nc.vector.tensor_copy(hcT_sb[:, 1, qsl], hc_acc[:, qsl])
        for tt in range(NTT):
            emit_oproj(1, tt, tail=True)

    nc.compile()
    return nc


# ---------------- host-side shard / unshard ----------------

_NC_CACHE = {}


def _get_nc():
    if "nc" not in _NC_CACHE:
        _NC_CACHE["nc"] = build_kernel()
    return _NC_CACHE["nc"]


def make_in_maps(q, k, v, Wq, bq, Wk, bk, Wv, bv, Wo, bo):
    bf = ml_dtypes.bfloat16
    maps = []
    qb = [np.ascontiguousarray(q[b].T.astype(bf)) for b in range(B)]
    kb_ = [np.ascontiguousarray(k[b].T.astype(bf)) for b in range(B)]
    vb = [np.ascontiguousarray(v[b].T.astype(bf)) for b in range(B)]
    for c in range(8):
        b = c // 4
        hc = c % 4
        cols = slice(256 * hc, 256 * hc + 256)
        wqkv = np.concatenate(
            [Wq[:, cols], Wk[:, cols], Wv[:, cols]], axis=1
        ).astype(bf)
        maps.append({
            "qT": qb[b],
            "kT": kb_[b],
            "vT": vb[b],
            "wqkv": np.ascontiguousarray(wqkv),
            "wo": np.ascontiguousarray(Wo[cols, :].astype(bf)),
            "bq": np.ascontiguousarray(
                bq[cols].reshape(NPAIR, 128, 1).astype(np.float32)
            ),
            "bk": np.ascontiguousarray(
                bk[cols].reshape(NPAIR, 128, 1).astype(np.float32)
            ),
            "bv": np.ascontiguousarray(bv[cols].astype(np.float32)),
        })
    return maps


def kernel(q, k, v, Wq, bq, Wk, bk, Wv, bv, Wo, bo):
    q = np.asarray(q, dtype=np.float32)
    k = np.asarray(k, dtype=np.float32)
    v = np.asarray(v, dtype=np.float32)
    Wq = np.asarray(Wq, dtype=np.float32)
    Wk = np.asarray(Wk, dtype=np.float32)
    Wv = np.asarray(Wv, dtype=np.float32)
    Wo = np.asarray(Wo, dtype=np.float32)
    bq = np.asarray(bq, dtype=np.float32)
    bk = np.asarray(bk, dtype=np.float32)
    bv = np.asarray(bv, dtype=np.float32)
    bo = np.asarray(bo, dtype=np.float32)

    nc = _get_nc()
    maps = make_in_maps(q, k, v, Wq, bq, Wk, bk, Wv, bv, Wo, bo)
    res = run_bass_kernel_spmd(nc, maps, core_ids=list(range(8)))

    outs = []
    for b in range(B):
        acc = np.zeros((S, D), dtype=np.float32)
        for hc in range(4):
            part = res.results[b * 4 + hc]["out"]  # [2, S, D] bf16
            acc += part[0].astype(np.float32)
            acc += part[1].astype(np.float32)
        acc += bo[None, :]
        outs.append(acc)
    return np.stack(outs, axis=0)


# revision 39
# speedup vs baseline: 1.2518x; 1.2518x over previous
"""Bass/Tile kernel for nn_MultiHeadAttention (B=2, S=2048, D=1024, H=16).

Sharding: 8 cores = 2 (batch) x 4 (head-chunks of 4 heads).
Each core computes, for its batch b and its 4 heads (2 pairs of 2 heads):
  qpT/kpT = (x @ W{q,k} + b)^T   in [dout, token] bf16 layout
  vp      = v @ Wv + bv          in [token, dout] bf16 layout
  scoresT = kp @ qp^T            per head, [k, q] f32 PSUM
  at      = exp(scoresT) bf16, Z[k] via ACT accum_out
  pv[qh]  = sum_kb (vp/Z)^T @ at  accumulated IN PSUM across all kb
  out[p]  = hcT_p^T @ Wo_p        per-pair bf16 partial (host sums 8 partials)

All matmuls bf16 (rel err ~1.25e-2 vs 2e-2 gate, matches numpy sim exactly).
v3: resident raw q/k/v tiles loaded with 2KB-per-partition DMA lines (2x DMA
efficiency vs 512-token chunks), packed wqkv weight tensor, early-exp pass
(scores for kb 0-3 on the first q-half start right after q-proj chunks 0/1 +
k-proj chunk 0), PV(kb-1) emitted between score groups to keep the PE stream
dense (DVFS ramp needs continuous tensor work), per-pair O-proj overlapped
with the other pair's attention.
"""

import sys

sys.path.insert(0, "/opt/trn_rl_repo")

from contextlib import ExitStack

import numpy as np
import ml_dtypes

import concourse.bass as bass
import concourse.mybir as mybir
import concourse.tile as tile
from concourse import bacc
from concourse.bass_utils import run_bass_kernel_spmd

BF16 = mybir.dt.bfloat16
F32 = mybir.dt.float32
AF = mybir.ActivationFunctionType
ALU = mybir.AluOpType

D = 1024
NK = 8  # k-tiles over D
DOUT = 256  # per-core head dims (4 heads)
NPAIR = 2  # pairs of heads (128 dout each)
HD = 64
S = 2048
B = 2
NKB = S // 128  # k-token blocks
NQH = S // 1024  # 1024-wide q halves
NTC = S // 512  # proj token chunks
NTT = S // 128  # token tiles
EARLY = 4  # kb blocks whose qh=0 scores/exp run during the projection head


def build_kernel():
    nc = bacc.Bacc("TRN2", target_bir_lowering=False, debug=False)

    qT = nc.dram_tensor("qT", [D, S], BF16, kind="ExternalInput")
    kT = nc.dram_tensor("kT", [D, S], BF16, kind="ExternalInput")
    vT = nc.dram_tensor("vT", [D, S], BF16, kind="ExternalInput")
    wqkv = nc.dram_tensor("wqkv", [D, 3 * DOUT], BF16, kind="ExternalInput")
    wo = nc.dram_tensor("wo", [DOUT, D], BF16, kind="ExternalInput")
    bq = nc.dram_tensor("bq", [NPAIR, 128, 1], F32, kind="ExternalInput")
    bk = nc.dram_tensor("bk", [NPAIR, 128, 1], F32, kind="ExternalInput")
    bv = nc.dram_tensor("bv", [DOUT], F32, kind="ExternalInput")
    out = nc.dram_tensor("out", [NPAIR, S, D], BF16, kind="ExternalOutput")

    qTv = qT.ap().rearrange("(t p) s -> t p s", p=128)  # [8, 128, S]
    kTv = kT.ap().rearrange("(t p) s -> t p s", p=128)
    vTv = vT.ap().rearrange("(t p) s -> t p s", p=128)
    wqkvv = wqkv.ap().rearrange("(t p) m -> t p m", p=128)  # [8, 128, 768]
    wov = wo.ap().rearrange("(t p) m -> t p m", p=128)  # [2, 128, 1024]
    bqv = bq.ap().rearrange("a p o -> p a o")  # [128, 2, 1]
    bkv = bk.ap().rearrange("a p o -> p a o")
    outv = out.ap().rearrange("a (t p) m -> a t p m", p=128)  # [2,16,128,1024]

    bv_bcast_ap = bass.AP(tensor=bv.ap().tensor, offset=0, ap=[[0, 128], [1, DOUT]])

    with tile.TileContext(nc) as tc, ExitStack() as ctx:
        sb = ctx.enter_context(tc.tile_pool(name="sb", bufs=1))

        # resident raw inputs + weights
        qT_sb = sb.tile([128, NK, S], BF16, tag="qT")
        kT_sb = sb.tile([128, NK, S], BF16, tag="kT")
        vT_sb = sb.tile([128, NK, S], BF16, tag="vT")
        wqkv_sb = sb.tile([128, NK, 3 * DOUT], BF16, tag="wqkv")
        wo_sb = sb.tile([128, NPAIR, D], BF16, tag="wo")
        bq_sb = sb.tile([128, NPAIR, 1], F32, tag="bq")
        bk_sb = sb.tile([128, NPAIR, 1], F32, tag="bk")
        bv_sb = sb.tile([128, DOUT], F32, tag="bv")

        # projection outputs (resident)
        qpT_sb = sb.tile([128, NPAIR, S], BF16, tag="qpT")
        kpT_sb = sb.tile([128, NPAIR, S], BF16, tag="kpT")
        vp_sb = sb.tile([128, NTT, DOUT], BF16, tag="vp")
        hcT_sb = sb.tile([128, NPAIR, S], BF16, tag="hcT")
        hc_acc = sb.tile([128, S], F32, tag="hc_acc")  # per-pair, reused

        psa = ctx.enter_context(tc.tile_pool(name="ps_all", bufs=1, space="PSUM"))
        asb = ctx.enter_context(tc.tile_pool(name="att_sb", bufs=1))
        osb = ctx.enter_context(tc.tile_pool(name="o_sb", bufs=1))

        # ---- DMA schedule: ordered by first consumption ----
        # wqkv[0] + qA first so the very first matmul can start early.
        nc.sync.dma_start(out=wqkv_sb[:, 0, :], in_=wqkvv[0])
        nc.sync.dma_start(out=bq_sb[:], in_=bqv)
        for kk in range(NK):  # q tokens 0..1023
            nc.sync.dma_start(out=qT_sb[:, kk, 0:1024], in_=qTv[kk][:, 0:1024])
        for kk in range(1, NK):
            nc.sync.dma_start(out=wqkv_sb[:, kk, :], in_=wqkvv[kk])
        nc.sync.dma_start(out=bk_sb[:], in_=bkv)
        nc.sync.dma_start(out=bv_sb[:], in_=bv_bcast_ap)
        for kk in range(NK):  # k tokens 0..1023
            nc.sync.dma_start(out=kT_sb[:, kk, 0:1024], in_=kTv[kk][:, 0:1024])
        for kk in range(NK):  # q tokens 1024..2047
            nc.sync.dma_start(out=qT_sb[:, kk, 1024:2048], in_=qTv[kk][:, 1024:2048])
        for kk in range(NK):  # v tokens 0..1023
            nc.sync.dma_start(out=vT_sb[:, kk, 0:1024], in_=vTv[kk][:, 0:1024])
        for kk in range(NK):  # k tokens 1024..2047
            nc.sync.dma_start(out=kT_sb[:, kk, 1024:2048], in_=kTv[kk][:, 1024:2048])
        for kk in range(NK):  # v tokens 1024..2047
            nc.sync.dma_start(out=vT_sb[:, kk, 1024:2048], in_=vTv[kk][:, 1024:2048])
        for t in range(NPAIR):
            nc.sync.dma_start(out=wo_sb[:, t, :], in_=wov[t])

        # ---- emission helpers ----
        BIGB = 4  # 'big' PSUM slots: 4 x [128,1024] f32 = all 8 banks

        def emit_qkproj_pair(XT_sb, woff, b_sb, XPT, tci, p):
            tsl = slice(tci * 512, tci * 512 + 512)
            ps_t = psa.tile([128, 512], F32, tag="big", bufs=BIGB, name=f"pj{p}")
            for kk in range(NK):
                nc.tensor.matmul(
                    ps_t[:],
                    lhsT=wqkv_sb[:, kk, woff + p * 128 : woff + p * 128 + 128],
                    rhs=XT_sb[:, kk, tsl],
                    start=(kk == 0),
                    stop=(kk == NK - 1),
                )
            nc.vector.tensor_scalar_add(XPT[:, p, tsl], ps_t[:], b_sb[:, p, :])

        def emit_vproj(tt):
            psv = psa.tile([128, DOUT], F32, tag="big", bufs=BIGB, name="projv")
            for kk in range(NK):
                nc.tensor.matmul(
                    psv[:],
                    lhsT=vT_sb[:, kk, tt * 128 : tt * 128 + 128],
                    rhs=wqkv_sb[:, kk, 2 * DOUT : 3 * DOUT],
                    start=(kk == 0),
                    stop=(kk == NK - 1),
                )
            nc.vector.scalar_tensor_tensor(
                out=vp_sb[:, tt, :],
                in0=psv[:],
                scalar=1.0,
                in1=bv_sb[:],
                op0=ALU.mult,
                op1=ALU.add,
            )

        def emit_oproj(p, tt, tail=False):
            ost = osb.tile([128, D], BF16, tag="ost", bufs=6, name="ost")
            pso = psa.tile([128, D], F32, tag="big", bufs=BIGB, name="o")
            for dc in range(2):
                nc.tensor.matmul(
                    pso[:, dc * 512 : dc * 512 + 512],
                    lhsT=hcT_sb[:, p, tt * 128 : tt * 128 + 128],
                    rhs=wo_sb[:, p, dc * 512 : dc * 512 + 512],
                    start=True,
                    stop=True,
                )
            if tail and tt % 2 == 0:
                nc.scalar.copy(ost[:], pso[:])
            else:
                nc.vector.tensor_copy(ost[:], pso[:])
            nc.sync.dma_start(out=outv[p][tt], in_=ost[:])

        def emit_sc_exp(p, kb, h, qh, z4):
            ksl = slice(kb * 128, kb * 128 + 128)
            hsl = slice(h * 64, h * 64 + 64)
            sc = psa.tile([128, 1024], F32, tag="big", bufs=BIGB, name=f"sc{h}{qh}")
            for qq in range(2):
                qsl = slice(qh * 1024 + qq * 512, qh * 1024 + qq * 512 + 512)
                nc.tensor.matmul(
                    sc[:, qq * 512 : qq * 512 + 512],
                    lhsT=kpT_sb[hsl, p, ksl],
                    rhs=qpT_sb[hsl, p, qsl],
                    start=True,
                    stop=True,
                )
            at = asb.tile([128, 1024], BF16, tag="at", bufs=14, name=f"at{h}{qh}")
            nc.scalar.activation(
                out=at[:], in_=sc[:], func=AF.Exp,
                accum_out=z4[:, h, qh : qh + 1],
            )
            return at

        def emit_zchain_h(p, kb, h, z4, vhs):
            # per-head z so the DVE work for head h issues as soon as that
            # head's two exps land (shortens the exp->vhs->PV ring)
            z1 = asb.tile([128, 1], F32, tag="z2", bufs=8, name="z1")
            nc.vector.tensor_add(z1[:], z4[:, h, 0:1], z4[:, h, 1:2])
            rz = asb.tile([128, 1], F32, tag="rz", bufs=8, name="rz")
            nc.vector.reciprocal(rz[:], z1[:])
            nc.vector.tensor_scalar_mul(
                vhs[:, h, :],
                vp_sb[:, kb, p * 128 + h * 64 : p * 128 + h * 64 + 64],
                rz[:],
            )

        def emit_pv_mm(ats, vhs):
            pvts = []
            for qh in range(NQH):
                pvt = psa.tile([128, 1024], F32, tag="big", bufs=BIGB, name="pvt")
                for h in range(2):
                    for qq in range(2):
                        nc.tensor.matmul(
                            pvt[h * 64 : h * 64 + 64, qq * 512 : qq * 512 + 512],
                            lhsT=vhs[:, h, :],
                            rhs=ats[(h, qh)][:, qq * 512 : qq * 512 + 512],
                            start=True,
                            stop=True,
                            tile_position=(0, h * 64),
                            skip_group_check=True,
                        )
                pvts.append(pvt)
            return pvts

        def emit_pv_acc(kb, pvts):
            # hc accumulation: emitted AFTER the z-chain so DVE's in-order
            # queue never blocks the exp-facing z work behind PE-dependent adds
            for qh in range(NQH):
                qsl = slice(qh * 1024, qh * 1024 + 1024)
                if kb == 0:
                    nc.vector.tensor_copy(hc_acc[:, qsl], pvts[qh][:])
                else:
                    nc.vector.tensor_add(hc_acc[:, qsl], hc_acc[:, qsl], pvts[qh][:])

        # ---- projection head + early exp pass (pair 0, kb<EARLY, qh=0) ----
        # only the chunks the early pass needs come first (pair-0 q chunks
        # 0/1 + pair-0 k chunk 0); every other chunk trickles later.
        emit_qkproj_pair(qT_sb, 0, bq_sb, qpT_sb, 0, 0)
        emit_qkproj_pair(qT_sb, 0, bq_sb, qpT_sb, 1, 0)
        emit_qkproj_pair(qT_sb, 0, bq_sb, qpT_sb, 0, 1)
        emit_qkproj_pair(qT_sb, 0, bq_sb, qpT_sb, 1, 1)
        emit_qkproj_pair(kT_sb, DOUT, bk_sb, kpT_sb, 0, 0)

        EARLY_TRICKLE = {
            0: (kT_sb, DOUT, bk_sb, kpT_sb, 1, 0),
            1: (kT_sb, DOUT, bk_sb, kpT_sb, 0, 1),
            2: (qT_sb, 0, bq_sb, qpT_sb, 2, 0),
            3: (qT_sb, 0, bq_sb, qpT_sb, 3, 0),
        }
        at_early = {}
        z4_early = {}
        for kb in range(EARLY):
            z4 = asb.tile([128, 2, NQH], F32, tag="z4e", bufs=EARLY + 1, name="z4e")
            z4_early[kb] = z4
            for h in range(2):
                at_early[(kb, h)] = emit_sc_exp(0, kb, h, 0, z4)
            emit_qkproj_pair(*EARLY_TRICKLE[kb])
        emit_vproj(0)
        emit_vproj(1)

        # ---- attention main loops ----
        # remaining projection chunks, scheduled just-in-time:
        # pair-0 k chunks c before kb=4c; all pair-1 chunks before iter 16
        MAIN_TRICKLE = {
            0: (qT_sb, 0, bq_sb, qpT_sb, 2, 1),
            1: (qT_sb, 0, bq_sb, qpT_sb, 3, 1),
            2: (kT_sb, DOUT, bk_sb, kpT_sb, 1, 1),
            4: (kT_sb, DOUT, bk_sb, kpT_sb, 2, 0),
            5: (kT_sb, DOUT, bk_sb, kpT_sb, 2, 1),
            8: (kT_sb, DOUT, bk_sb, kpT_sb, 3, 0),
            9: (kT_sb, DOUT, bk_sb, kpT_sb, 3, 1),
        }
        # fused loop over (pair, kb): the pair transition pipelines like any
        # other iteration (prev iteration's PV rides through the next one)
        prev = None  # (kb, ats, vhs)
        for it in range(NPAIR * NKB):
            p, kb = it // NKB, it % NKB
            vhs = asb.tile([128, 2, HD], BF16, tag="vhs", bufs=8, name="vhs")
            if p == 0 and kb < EARLY:
                z4 = z4_early[kb]
                ats = {
                    (0, 0): at_early[(kb, 0)],
                    (1, 0): at_early[(kb, 1)],
                }
                ats[(0, 1)] = emit_sc_exp(p, kb, 0, 1, z4)
                emit_zchain_h(p, kb, 0, z4, vhs)
                ats[(1, 1)] = emit_sc_exp(p, kb, 1, 1, z4)
                emit_zchain_h(p, kb, 1, z4, vhs)
            else:
                z4 = asb.tile([128, 2, NQH], F32, tag="z4", bufs=8, name="z4")
                ats = {}
                ats[(0, 0)] = emit_sc_exp(p, kb, 0, 0, z4)
                ats[(0, 1)] = emit_sc_exp(p, kb, 0, 1, z4)
                emit_zchain_h(p, kb, 0, z4, vhs)
                ats[(1, 0)] = emit_sc_exp(p, kb, 1, 0, z4)
                ats[(1, 1)] = emit_sc_exp(p, kb, 1, 1, z4)
                emit_zchain_h(p, kb, 1, z4, vhs)
            if prev is not None:
                pvts = emit_pv_mm(prev[1], prev[2])
                emit_pv_acc(prev[0], pvts)
                if prev[0] == NKB - 1:  # pair-0 finished: stage its hcT
                    for qh in range(NQH):
                        qsl = slice(qh * 1024, qh * 1024 + 1024)
                        nc.vector.tensor_copy(hcT_sb[:, 0, qsl], hc_acc[:, qsl])
            prev = (kb, ats, vhs)
            # trickled, dependency-free PE filler
            if p == 0:
                if kb < 14:
                    emit_vproj(kb + 2)
                tr = MAIN_TRICKLE.get(kb)
                if tr is not None:
                    emit_qkproj_pair(*tr)
            elif kb >= 1:
                emit_oproj(0, kb - 1)

        # ---- drain: last PV, pair-1 hcT, remaining O-projection ----
        pvts = emit_pv_mm(prev[1], prev[2])
        emit_pv_acc(NKB - 1, pvts)
        for qh in range(NQH):
            qsl = slice(qh * 1024, qh * 1024 + 1024)
            nc.vector.tensor_copy(hcT_sb[:, 1, qsl], hc_acc[:, qsl])
        emit_oproj(0, NTT - 1)
        for tt in range(NTT):
            emit_oproj(1, tt, tail=True)

    nc.compile()
    return nc


# ---------------- host-side shard / unshard ----------------

_NC_CACHE = {}


def _get_nc():
    if "nc" not in _NC_CACHE:
        _NC_CACHE["nc"] = build_kernel()
    return _NC_CACHE["nc"]


def make_in_maps(q, k, v, Wq, bq, Wk, bk, Wv, bv, Wo, bo):
    bf = ml_dtypes.bfloat16
    maps = []
    qb = [np.ascontiguousarray(q[b].T.astype(bf)) for b in range(B)]
    kb_ = [np.ascontiguousarray(k[b].T.astype(bf)) for b in range(B)]
    vb = [np.ascontiguousarray(v[b].T.astype(bf)) for b in range(B)]
    for c in range(8):
        b = c // 4
        hc = c % 4
        cols = slice(256 * hc, 256 * hc + 256)
        wqkv = np.concatenate(
            [Wq[:, cols], Wk[:, cols], Wv[:, cols]], axis=1
        ).astype(bf)
        maps.append({
            "qT": qb[b],
            "kT": kb_[b],
            "vT": vb[b],
            "wqkv": np.ascontiguousarray(wqkv),
            "wo": np.ascontiguousarray(Wo[cols, :].astype(bf)),
            "bq": np.ascontiguousarray(
                bq[cols].reshape(NPAIR, 128, 1).astype(np.float32)
            ),
            "bk": np.ascontiguousarray(
                bk[cols].reshape(NPAIR, 128, 1).astype(np.float32)
            ),
            "bv": np.ascontiguousarray(bv[cols].astype(np.float32)),
        })
    return maps


def kernel(q, k, v, Wq, bq, Wk, bk, Wv, bv, Wo, bo):
    q = np.asarray(q, dtype=np.float32)
    k = np.asarray(k, dtype=np.float32)
    v = np.asarray(v, dtype=np.float32)
    Wq = np.asarray(Wq, dtype=np.float32)
    Wk = np.asarray(Wk, dtype=np.float32)
    Wv = np.asarray(Wv, dtype=np.float32)
    Wo = np.asarray(Wo, dtype=np.float32)
    bq = np.asarray(bq, dtype=np.float32)
    bk = np.asarray(bk, dtype=np.float32)
    bv = np.asarray(bv, dtype=np.float32)
    bo = np.asarray(bo, dtype=np.float32)

    nc = _get_nc()
    maps = make_in_maps(q, k, v, Wq, bq, Wk, bk, Wv, bv, Wo, bo)
    res = run_bass_kernel_spmd(nc, maps, core_ids=list(range(8)))

    outs = []
    for b in range(B):
        acc = np.zeros((S, D), dtype=np.float32)
        for hc in range(4):
            part = res.results[b * 4 + hc]["out"]  # [2, S, D] bf16
            acc += part[0].astype(np.float32)
            acc += part[1].astype(np.float32)
        acc += bo[None, :]
        outs.append(acc)
    return np.stack(outs, axis=0)


# revision 40
# speedup vs baseline: 1.2617x; 1.0079x over previous
"""Bass/Tile kernel for nn_MultiHeadAttention (B=2, S=2048, D=1024, H=16).

Sharding: 8 cores = 2 (batch) x 4 (head-chunks of 4 heads).
Each core computes, for its batch b and its 4 heads (2 pairs of 2 heads):
  qpT/kpT = (x @ W{q,k} + b)^T   in [dout, token] bf16 layout
  vp      = v @ Wv + bv          in [token, dout] bf16 layout
  scoresT = kp @ qp^T            per head, [k, q] f32 PSUM
  at      = exp(scoresT) bf16, Z[k] via ACT accum_out
  pv[qh]  = sum_kb (vp/Z)^T @ at  accumulated IN PSUM across all kb
  out[p]  = hcT_p^T @ Wo_p        per-pair bf16 partial (host sums 8 partials)

All matmuls bf16 (rel err ~1.25e-2 vs 2e-2 gate, matches numpy sim exactly).
v3: resident raw q/k/v tiles loaded with 2KB-per-partition DMA lines (2x DMA
efficiency vs 512-token chunks), packed wqkv weight tensor, early-exp pass
(scores for kb 0-3 on the first q-half start right after q-proj chunks 0/1 +
k-proj chunk 0), PV(kb-1) emitted between score groups to keep the PE stream
dense (DVFS ramp needs continuous tensor work), per-pair O-proj overlapped
with the other pair's attention.
"""

import sys

sys.path.insert(0, "/opt/trn_rl_repo")

from contextlib import ExitStack

import numpy as np
import ml_dtypes

import concourse.bass as bass
import concourse.mybir as mybir
import concourse.tile as tile
from concourse import bacc
from concourse.bass_utils import run_bass_kernel_spmd

BF16 = mybir.dt.bfloat16
F32 = mybir.dt.float32
AF = mybir.ActivationFunctionType
ALU = mybir.AluOpType

D = 1024
NK = 8  # k-tiles over D
DOUT = 256  # per-core head dims (4 heads)
NPAIR = 2  # pairs of heads (128 dout each)
HD = 64
S = 2048
B = 2
NKB = S // 128  # k-token blocks
NQH = S // 1024  # 1024-wide q halves
NTC = S // 512  # proj token chunks
NTT = S // 128  # token tiles
EARLY = 4  # kb blocks whose qh=0 scores/exp run during the projection head


def build_kernel():
    nc = bacc.Bacc("TRN2", target_bir_lowering=False, debug=False)

    qT = nc.dram_tensor("qT", [D, S], BF16, kind="ExternalInput")
    kT = nc.dram_tensor("kT", [D, S], BF16, kind="ExternalInput")
    vT = nc.dram_tensor("vT", [D, S], BF16, kind="ExternalInput")
    wqkv = nc.dram_tensor("wqkv", [D, 3 * DOUT], BF16, kind="ExternalInput")
    wo = nc.dram_tensor("wo", [DOUT, D], BF16, kind="ExternalInput")
    bq = nc.dram_tensor("bq", [NPAIR, 128, 1], F32, kind="ExternalInput")
    bk = nc.dram_tensor("bk", [NPAIR, 128, 1], F32, kind="ExternalInput")
    bv = nc.dram_tensor("bv", [DOUT], F32, kind="ExternalInput")
    out = nc.dram_tensor("out", [NPAIR, S, D], BF16, kind="ExternalOutput")

    qTv = qT.ap().rearrange("(t p) s -> t p s", p=128)  # [8, 128, S]
    kTv = kT.ap().rearrange("(t p) s -> t p s", p=128)
    vTv = vT.ap().rearrange("(t p) s -> t p s", p=128)
    wqkvv = wqkv.ap().rearrange("(t p) m -> t p m", p=128)  # [8, 128, 768]
    wov = wo.ap().rearrange("(t p) m -> t p m", p=128)  # [2, 128, 1024]
    bqv = bq.ap().rearrange("a p o -> p a o")  # [128, 2, 1]
    bkv = bk.ap().rearrange("a p o -> p a o")
    outv = out.ap().rearrange("a (t p) m -> a t p m", p=128)  # [2,16,128,1024]

    bv_bcast_ap = bass.AP(tensor=bv.ap().tensor, offset=0, ap=[[0, 128], [1, DOUT]])

    with tile.TileContext(nc) as tc, ExitStack() as ctx:
        sb = ctx.enter_context(tc.tile_pool(name="sb", bufs=1))

        # resident raw inputs + weights
        qT_sb = sb.tile([128, NK, S], BF16, tag="qT")
        kT_sb = sb.tile([128, NK, S], BF16, tag="kT")
        vT_sb = sb.tile([128, NK, S], BF16, tag="vT")
        wqkv_sb = sb.tile([128, NK, 3 * DOUT], BF16, tag="wqkv")
        wo_sb = sb.tile([128, NPAIR, D], BF16, tag="wo")
        bq_sb = sb.tile([128, NPAIR, 1], F32, tag="bq")
        bk_sb = sb.tile([128, NPAIR, 1], F32, tag="bk")
        bv_sb = sb.tile([128, DOUT], F32, tag="bv")

        # projection outputs (resident)
        qpT_sb = sb.tile([128, NPAIR, S], BF16, tag="qpT")
        kpT_sb = sb.tile([128, NPAIR, S], BF16, tag="kpT")
        vp_sb = sb.tile([128, NTT, DOUT], BF16, tag="vp")
        hcT_sb = sb.tile([128, NPAIR, S], BF16, tag="hcT")
        hc_acc = sb.tile([128, S], F32, tag="hc_acc")  # per-pair, reused

        psa = ctx.enter_context(tc.tile_pool(name="ps_all", bufs=1, space="PSUM"))
        asb = ctx.enter_context(tc.tile_pool(name="att_sb", bufs=1))
        osb = ctx.enter_context(tc.tile_pool(name="o_sb", bufs=1))

        # ---- DMA schedule: ordered by first consumption ----
        # wqkv[0] + qA first so the very first matmul can start early.
        nc.sync.dma_start(out=wqkv_sb[:, 0, :], in_=wqkvv[0])
        nc.sync.dma_start(out=bq_sb[:], in_=bqv)
        for kk in range(NK):  # q tokens 0..1023
            nc.sync.dma_start(out=qT_sb[:, kk, 0:1024], in_=qTv[kk][:, 0:1024])
        for kk in range(1, NK):
            nc.sync.dma_start(out=wqkv_sb[:, kk, :], in_=wqkvv[kk])
        nc.sync.dma_start(out=bk_sb[:], in_=bkv)
        nc.sync.dma_start(out=bv_sb[:], in_=bv_bcast_ap)
        for kk in range(NK):  # k tokens 0..1023
            nc.sync.dma_start(out=kT_sb[:, kk, 0:1024], in_=kTv[kk][:, 0:1024])
        for kk in range(NK):  # q tokens 1024..2047
            nc.sync.dma_start(out=qT_sb[:, kk, 1024:2048], in_=qTv[kk][:, 1024:2048])
        for kk in range(NK):  # v tokens 0..1023
            nc.sync.dma_start(out=vT_sb[:, kk, 0:1024], in_=vTv[kk][:, 0:1024])
        for kk in range(NK):  # k tokens 1024..2047
            nc.sync.dma_start(out=kT_sb[:, kk, 1024:2048], in_=kTv[kk][:, 1024:2048])
        for kk in range(NK):  # v tokens 1024..2047
            nc.sync.dma_start(out=vT_sb[:, kk, 1024:2048], in_=vTv[kk][:, 1024:2048])
        for t in range(NPAIR):
            nc.sync.dma_start(out=wo_sb[:, t, :], in_=wov[t])

        # ---- emission helpers ----
        BIGB = 4  # 'big' PSUM slots: 4 x [128,1024] f32 = all 8 banks

        def emit_qkproj_pair(XT_sb, woff, b_sb, XPT, tci, p):
            tsl = slice(tci * 512, tci * 512 + 512)
            ps_t = psa.tile([128, 512], F32, tag="big", bufs=BIGB, name=f"pj{p}")
            for kk in range(NK):
                nc.tensor.matmul(
                    ps_t[:],
                    lhsT=wqkv_sb[:, kk, woff + p * 128 : woff + p * 128 + 128],
                    rhs=XT_sb[:, kk, tsl],
                    start=(kk == 0),
                    stop=(kk == NK - 1),
                )
            nc.vector.tensor_scalar_add(XPT[:, p, tsl], ps_t[:], b_sb[:, p, :])

        def emit_vproj(tt):
            psv = psa.tile([128, DOUT], F32, tag="big", bufs=BIGB, name="projv")
            for kk in range(NK):
                nc.tensor.matmul(
                    psv[:],
                    lhsT=vT_sb[:, kk, tt * 128 : tt * 128 + 128],
                    rhs=wqkv_sb[:, kk, 2 * DOUT : 3 * DOUT],
                    start=(kk == 0),
                    stop=(kk == NK - 1),
                )
            nc.vector.scalar_tensor_tensor(
                out=vp_sb[:, tt, :],
                in0=psv[:],
                scalar=1.0,
                in1=bv_sb[:],
                op0=ALU.mult,
                op1=ALU.add,
            )

        def emit_oproj(p, tt, tail=False):
            ost = osb.tile([128, D], BF16, tag="ost", bufs=6, name="ost")
            pso = psa.tile([128, D], F32, tag="big", bufs=BIGB, name="o")
            for dc in range(2):
                nc.tensor.matmul(
                    pso[:, dc * 512 : dc * 512 + 512],
                    lhsT=hcT_sb[:, p, tt * 128 : tt * 128 + 128],
                    rhs=wo_sb[:, p, dc * 512 : dc * 512 + 512],
                    start=True,
                    stop=True,
                )
            if tail and tt % 2 == 0:
                nc.scalar.copy(ost[:], pso[:])
            else:
                nc.vector.tensor_copy(ost[:], pso[:])
            nc.sync.dma_start(out=outv[p][tt], in_=ost[:])

        def emit_sc_exp(p, kb, h, qh, z4):
            ksl = slice(kb * 128, kb * 128 + 128)
            hsl = slice(h * 64, h * 64 + 64)
            sc = psa.tile([128, 1024], F32, tag="big", bufs=BIGB, name=f"sc{h}{qh}")
            for qq in range(2):
                qsl = slice(qh * 1024 + qq * 512, qh * 1024 + qq * 512 + 512)
                nc.tensor.matmul(
                    sc[:, qq * 512 : qq * 512 + 512],
                    lhsT=kpT_sb[hsl, p, ksl],
                    rhs=qpT_sb[hsl, p, qsl],
                    start=True,
                    stop=True,
                )
            at = asb.tile([128, 1024], BF16, tag="at", bufs=14, name=f"at{h}{qh}")
            nc.scalar.activation(
                out=at[:], in_=sc[:], func=AF.Exp,
                accum_out=z4[:, h, qh : qh + 1],
            )
            return at

        def emit_zchain_h(p, kb, h, z4, vhs):
            # per-head z so the DVE work for head h issues as soon as that
            # head's two exps land (shortens the exp->vhs->PV ring)
            z1 = asb.tile([128, 1], F32, tag="z2", bufs=8, name="z1")
            nc.vector.tensor_add(z1[:], z4[:, h, 0:1], z4[:, h, 1:2])
            rz = asb.tile([128, 1], F32, tag="rz", bufs=8, name="rz")
            nc.vector.reciprocal(rz[:], z1[:])
            nc.vector.tensor_scalar_mul(
                vhs[:, h, :],
                vp_sb[:, kb, p * 128 + h * 64 : p * 128 + h * 64 + 64],
                rz[:],
            )

        def emit_pv_mm(ats, vhs):
            pvts = []
            for qh in range(NQH):
                pvt = psa.tile([128, 1024], F32, tag="big", bufs=BIGB, name="pvt")
                for h in range(2):
                    for qq in range(2):
                        nc.tensor.matmul(
                            pvt[h * 64 : h * 64 + 64, qq * 512 : qq * 512 + 512],
                            lhsT=vhs[:, h, :],
                            rhs=ats[(h, qh)][:, qq * 512 : qq * 512 + 512],
                            start=True,
                            stop=True,
                            tile_position=(0, h * 64),
                            skip_group_check=True,
                        )
                pvts.append(pvt)
            return pvts

        def emit_pv_acc(kb, pvts):
            # hc accumulation: emitted AFTER the z-chain so DVE's in-order
            # queue never blocks the exp-facing z work behind PE-dependent adds
            for qh in range(NQH):
                qsl = slice(qh * 1024, qh * 1024 + 1024)
                if kb == 0:
                    nc.vector.tensor_copy(hc_acc[:, qsl], pvts[qh][:])
                else:
                    nc.vector.tensor_add(hc_acc[:, qsl], hc_acc[:, qsl], pvts[qh][:])

        # ---- projection head + early exp pass (pair 0, kb<EARLY, qh=0) ----
        # only the chunks the early pass needs come first (pair-0 q chunks
        # 0/1 + pair-0 k chunk 0); every other chunk trickles later.
        emit_qkproj_pair(qT_sb, 0, bq_sb, qpT_sb, 0, 0)
        emit_qkproj_pair(qT_sb, 0, bq_sb, qpT_sb, 1, 0)
        emit_qkproj_pair(qT_sb, 0, bq_sb, qpT_sb, 0, 1)
        emit_qkproj_pair(qT_sb, 0, bq_sb, qpT_sb, 1, 1)
        emit_qkproj_pair(kT_sb, DOUT, bk_sb, kpT_sb, 0, 0)

        EARLY_TRICKLE = {
            0: (kT_sb, DOUT, bk_sb, kpT_sb, 1, 0),
            1: (kT_sb, DOUT, bk_sb, kpT_sb, 0, 1),
            2: (qT_sb, 0, bq_sb, qpT_sb, 2, 0),
            3: (qT_sb, 0, bq_sb, qpT_sb, 3, 0),
        }
        at_early = {}
        z4_early = {}
        for kb in range(EARLY):
            z4 = asb.tile([128, 2, NQH], F32, tag="z4e", bufs=EARLY + 1, name="z4e")
            z4_early[kb] = z4
            for h in range(2):
                at_early[(kb, h)] = emit_sc_exp(0, kb, h, 0, z4)
            emit_qkproj_pair(*EARLY_TRICKLE[kb])
        emit_vproj(0)
        emit_vproj(1)

        # ---- attention main loops ----
        # remaining projection chunks, scheduled just-in-time:
        # pair-0 k chunks c before kb=4c; all pair-1 chunks before iter 16
        MAIN_TRICKLE = {
            0: (qT_sb, 0, bq_sb, qpT_sb, 2, 1),
            1: (qT_sb, 0, bq_sb, qpT_sb, 3, 1),
            2: (kT_sb, DOUT, bk_sb, kpT_sb, 1, 1),
            4: (kT_sb, DOUT, bk_sb, kpT_sb, 2, 0),
            5: (kT_sb, DOUT, bk_sb, kpT_sb, 2, 1),
            8: (kT_sb, DOUT, bk_sb, kpT_sb, 3, 0),
            9: (kT_sb, DOUT, bk_sb, kpT_sb, 3, 1),
        }
        # fused loop over (pair, kb): the pair transition pipelines like any
        # other iteration (prev iteration's PV rides through the next one)
        prev = None  # (kb, ats, vhs)
        for it in range(NPAIR * NKB):
            p, kb = it // NKB, it % NKB
            vhs = asb.tile([128, 2, HD], BF16, tag="vhs", bufs=8, name="vhs")
            if p == 0 and kb < EARLY:
                z4 = z4_early[kb]
                ats = {
                    (0, 0): at_early[(kb, 0)],
                    (1, 0): at_early[(kb, 1)],
                }
                ats[(0, 1)] = emit_sc_exp(p, kb, 0, 1, z4)
                emit_zchain_h(p, kb, 0, z4, vhs)
                ats[(1, 1)] = emit_sc_exp(p, kb, 1, 1, z4)
                emit_zchain_h(p, kb, 1, z4, vhs)
            else:
                z4 = asb.tile([128, 2, NQH], F32, tag="z4", bufs=8, name="z4")
                ats = {}
                ats[(0, 0)] = emit_sc_exp(p, kb, 0, 0, z4)
                ats[(0, 1)] = emit_sc_exp(p, kb, 0, 1, z4)
                emit_zchain_h(p, kb, 0, z4, vhs)
                ats[(1, 0)] = emit_sc_exp(p, kb, 1, 0, z4)
                ats[(1, 1)] = emit_sc_exp(p, kb, 1, 1, z4)
                emit_zchain_h(p, kb, 1, z4, vhs)
            if prev is not None:
                pvts = emit_pv_mm(prev[1], prev[2])
                emit_pv_acc(prev[0], pvts)
                if prev[0] == NKB - 1:  # pair-0 finished: stage its hcT
                    for qh in range(NQH):
                        qsl = slice(qh * 1024, qh * 1024 + 1024)
                        nc.vector.tensor_copy(hcT_sb[:, 0, qsl], hc_acc[:, qsl])
            prev = (kb, ats, vhs)
            # trickled, dependency-free PE filler
            if p == 0:
                if kb < 14:
                    emit_vproj(kb + 2)
                tr = MAIN_TRICKLE.get(kb)
                if tr is not None:
                    emit_qkproj_pair(*tr)
            elif kb >= 1:
                emit_oproj(0, kb - 1)

        # ---- drain: last PV, pair-1 hcT, remaining O-projection ----
        # oproj(0,15) first: PE has ready work while DVE drains the last PV;
        # the two hcT casts go to different engines so they overlap.
        pvts = emit_pv_mm(prev[1], prev[2])
        emit_oproj(0, NTT - 1)
        emit_pv_acc(NKB - 1, pvts)
        nc.vector.tensor_copy(hcT_sb[:, 1, 0:1024], hc_acc[:, 0:1024])
        nc.scalar.copy(hcT_sb[:, 1, 1024:2048], hc_acc[:, 1024:2048])
        for tt in range(NTT):
            emit_oproj(1, tt, tail=True)

    nc.compile()
    return nc


# ---------------- host-side shard / unshard ----------------

_NC_CACHE = {}


def _get_nc():
    if "nc" not in _NC_CACHE:
        _NC_CACHE["nc"] = build_kernel()
    return _NC_CACHE["nc"]


def make_in_maps(q, k, v, Wq, bq, Wk, bk, Wv, bv, Wo, bo):
    bf = ml_dtypes.bfloat16
    maps = []
    qb = [np.ascontiguousarray(q[b].T.astype(bf)) for b in range(B)]
    kb_ = [np.ascontiguousarray(k[b].T.astype(bf)) for b in range(B)]
    vb = [np.ascontiguousarray(v[b].T.astype(bf)) for b in range(B)]
    for c in range(8):
        b = c // 4
        hc = c % 4
        cols = slice(256 * hc, 256 * hc + 256)
        wqkv = np.concatenate(
            [Wq[:, cols], Wk[:, cols], Wv[:, cols]], axis=1
        ).astype(bf)
        maps.append({
            "qT": qb[b],
            "kT": kb_[b],
            "vT": vb[b],
            "wqkv": np.ascontiguousarray(wqkv),
            "wo": np.ascontiguousarray(Wo[cols, :].astype(bf)),
            "bq": np.ascontiguousarray(
                bq[cols].reshape(NPAIR, 128, 1).astype(np.float32)
            ),
            "bk": np.ascontiguousarray(
                bk[cols].reshape(NPAIR, 128, 1).astype(np.float32)
            ),
            "bv": np.ascontiguousarray(bv[cols].astype(np.float32)),
        })
    return maps


def kernel(q, k, v, Wq, bq, Wk, bk, Wv, bv, Wo, bo):
    q = np.asarray(q, dtype=np.float32)
    k = np.asarray(k, dtype=np.float32)
    v = np.asarray(v, dtype=np.float32)
    Wq = np.asarray(Wq, dtype=np.float32)
    Wk = np.asarray(Wk, dtype=np.float32)
    Wv = np.asarray(Wv, dtype=np.float32)
    Wo = np.asarray(Wo, dtype=np.float32)
    bq = np.asarray(bq, dtype=np.float32)
    bk = np.asarray(bk, dtype=np.float32)
    bv = np.asarray(bv, dtype=np.float32)
    bo = np.asarray(bo, dtype=np.float32)

    nc = _get_nc()
    maps = make_in_maps(q, k, v, Wq, bq, Wk, bk, Wv, bv, Wo, bo)
    res = run_bass_kernel_spmd(nc, maps, core_ids=list(range(8)))

    outs = []
    for b in range(B):
        acc = np.zeros((S, D), dtype=np.float32)
        for hc in range(4):
            part = res.results[b * 4 + hc]["out"]  # [2, S, D] bf16
            acc += part[0].astype(np.float32)
            acc += part[1].astype(np.float32)
        acc += bo[None, :]
        outs.append(acc)
    return np.stack(outs, axis=0)
